# revision 31
# baseline (speedup 1.0000x reference)
"""Causal self-attention (B=2, T=2048, C=768, H=12) on 8 TRN2 NeuronCores.

Sharding: 24 (batch, head) pairs -> 8 cores x 3 heads (head-tensor-parallel
within a batch, data-parallel across the 2 batches: cores 0-3 = batch 0,
cores 4-7 = batch 1). Each core computes qkv for its 3 heads, causal
attention, and a rank-192 partial of the output projection; the host sums
the 4 partials per batch and adds b_proj.

v2 (bf16 datapath): all matmul operands are bf16 (inputs pre-rounded on the
host; PSUM stays f32), halving input DMA and SBUF pressure and making the
<256-wide diagonal matmuls full-rate. qkv is emitted nch-outer (all 5 row
groups per 512-column x^T chunk) and interleaved into the h0/h1 attention
rounds so the attention exp stream (ACT engine) starts ~20us earlier and the
input-DMA window never starves the PE:

  round -1: qkv chunk 0 (all groups) + v-tile transposes 0..3
  round j in 0..3: attn(h0,j) + attn(h1,j) + qkv chunk j+1 (if any)
  tail:     attn(h2,j) for j=0..3 with proj tiles pipelined per chunk

Scores stay transposed (S^T[k,q] per 128-row k-tile) so softmax needs no max
subtraction and O^T accumulates over k-tiles in PSUM; the softmax denominator
falls out of the same matmul via a ones-column appended to the v stationary
(65th output row). Causal mask: gpsimd zeroes the invalid triangle of exp(S)
on diagonal tiles only. PSUM: S 3x[128,1024] + O 2x1bank = 8 banks.
"""

import os
import sys

for _p in ("/opt/trn_rl_repo", "/root/.axon_site/_ro/trn_rl_repo"):
    if os.path.isdir(_p) and _p not in sys.path:
        sys.path.insert(0, _p)

import numpy as np

import concourse.bass as bass  # noqa: F401
import concourse.mybir as mybir
import concourse.tile as tile
from concourse import bacc
from concourse.bass_utils import run_bass_kernel_spmd
from concourse.masks import make_identity

B, T, C, H, DH = 2, 2048, 768, 12, 64
HPC = 3          # heads per core
NCORES = 8
KO = C // 128    # 6 contraction tiles over the model dim
F32 = mybir.dt.float32
F32R = mybir.dt.float32r
BF16 = mybir.dt.bfloat16
AF = mybir.ActivationFunctionType
ALU = mybir.AluOpType
E_BUFS, N_BUFS, Y_BUFS, S_BUFS, O_BUFS = 6, 4, 4, 4, 3
WARMUP_MM = 6

# qkv row groups: G0=[q0|q1] G1=[k0|k1] G2=[v0|v1] G3=[q2|v2] G4=[k2|pad]
# (matmul requires lhsT/rhs at the same partition base, so each head's q and
# k must share a base: h0/h2 at base 0, h1 at base 64)
Q_POS = {0: (0, 0), 1: (0, 64), 2: (3, 0)}
K_POS = {0: (1, 0), 1: (1, 64), 2: (4, 0)}
V_POS = {0: (2, 0), 1: (2, 64), 2: (3, 64)}
# host-side column order matching the groups ((kind 0=q/1=k/2=v, head idx))
W_ORDER = [(0, 0), (0, 1), (1, 0), (1, 1), (2, 0), (2, 1), (0, 2), (2, 2),
           (1, 2)]


def to_f32r(a):
    """Round fp32 to the fp32r wire format: 8-bit exponent, 11-bit mantissa
    (round-to-nearest-even), low 12 mantissa bits zero. Matches walrus's
    fp32_to_fp32r; required for data consumed directly by fp32r matmuls."""
    u = np.ascontiguousarray(a, np.float32).view(np.uint32).copy()
    low = u & np.uint32(0xFFF)
    lsb = (u >> np.uint32(12)) & np.uint32(1)
    add = ((low > 0x800) | ((low == 0x800) & (lsb == 1))).astype(np.uint32) << 12
    u = ((u & ~np.uint32(0xFFF)) + add).astype(np.uint32)
    return u.view(np.float32)


def _build_body(nc, tc, xt_d, w_d, b_d, wp_d, y_d, phases=('qkv', 'attn', 'proj')):
    with (
        tc.tile_pool(name="persist", bufs=1) as pp,
        tc.tile_pool(name="sb_att", bufs=E_BUFS) as sbE,
        tc.tile_pool(name="sb_n", bufs=N_BUFS) as sbN,
        tc.tile_pool(name="sb_y", bufs=Y_BUFS) as sbY,
        tc.tile_pool(name="psum", bufs=1, space="PSUM") as psp,
    ):
        # -- constants ----------------------------------------------------
        ident = pp.tile([128, 128], F32, name="ident")
        make_identity(nc, ident)
        # causal triangle: tri[p, c] = 1 where c >= p else 0 (applied to the
        # 128-wide diagonal blocks of exp(S^T) by a DVE multiply)
        tri = pp.tile([128, 128], F32, name="tri")
        nc.gpsimd.memset(tri, 1.0)
        nc.gpsimd.affine_select(
            out=tri, in_=tri, compare_op=ALU.is_ge, fill=0.0,
            base=0, pattern=[[1, 128]], channel_multiplier=-1,
        )

        # -- PE warmup: keep the HAM activity monitor busy while the input
        # DMAs land so real matmuls start at 2.4 GHz, not 1.2 --------------
        warm = psp.tile([128, 512], F32, name="warm", tag="S", bufs=S_BUFS)
        for wi in range(WARMUP_MM):
            nc.tensor.matmul(warm[:, 0:128], ident, ident,
                             start=True, stop=True, skip_group_check=True)

        # -- input DMAs (bias+weights first -- the first qkv matmul needs
        # w_sb and xt chunk 0; wp last: only needed by the projection) ----
        b_sb = pp.tile([128, 5], F32, name="b_sb")
        nc.sync.dma_start(b_sb[:], b_d[:])
        # w and x^T chunk 0 split per contraction tile so the first qkv
        # matmul chain can start after ~1us instead of waiting for both
        # tensors in full; later x^T chunks land well ahead of use
        w_sb = pp.tile([128, KO, 576], F32R, name="w_sb")
        w_r = w_d.rearrange("(ko p) m -> p ko m", p=128)
        xt_sb = pp.tile([128, KO, T], F32R, name="xt_sb")
        xt_r = xt_d.rearrange("(ko p) n -> p ko n", p=128)
        for ko in range(KO):
            nc.sync.dma_start(w_sb[:, ko, :], w_r[:, ko, :])
            nc.sync.dma_start(xt_sb[:, ko, 0:512], xt_r[:, ko, 0:512])
        for nch in range(1, 4):
            nc.sync.dma_start(
                xt_sb[:, :, 512 * nch:512 * (nch + 1)],
                xt_r[:, :, 512 * nch:512 * (nch + 1)],
            )
        wpa = pp.tile([128, C], F32R, name="wpa")
        nc.sync.dma_start(wpa, wp_d[0:128, :])
        wpb = pp.tile([64, C], F32R, name="wpb")
        nc.sync.dma_start(wpb, wp_d[128:192, :])

        qkvT = pp.tile([128, 5, T], F32R, name="qkvT")
        ones_col = pp.tile([128, 1], F32R, name="ones_col")
        nc.scalar.activation(ones_col, b_sb[:, 0:1], AF.Copy, bias=1.0, scale=0.0)
        v_sb = [pp.tile([128, 16, 65], F32R, name=f"v_sb{h}") for h in range(HPC)]
        for h in range(HPC):
            nc.vector.tensor_copy(v_sb[h][:, :, 64:65],
                                  ones_col.broadcast_to([128, 16, 1]))
        OT_a = pp.tile([128, T], F32R, name="OT_a")   # heads 0,1 of O^T
        OT_b = pp.tile([64, T], F32R, name="OT_b")    # head 2
        h1tmp = pp.tile([64, T], F32R, name="h1tmp")  # head 1 staging

        def s_tile(name):
            return psp.tile([128, 512], F32, name=name, tag="S", bufs=S_BUFS)

        # -- qkv: per 512-col x^T chunk, all 5 groups + the v transposes
        #    that chunk enables; emitted as a generator for interleaving --
        def transpose_v_kt(h, kt, par):
            g, r0 = V_POS[h]
            vT = qkvT[r0:r0 + 64, g, 128 * kt:128 * (kt + 1)]
            tp = s_tile("tp_ps")
            nc.tensor.transpose(tp[:, 0:64], vT.bitcast(F32),
                                ident[r0:r0 + 64, r0:r0 + 64])
            # gpsimd cannot read PSUM: v copies stay on DVE
            nc.vector.tensor_copy(v_sb[h][:, kt, 0:64], tp[:, 0:64])

        def qkv_groups(nch, groups, off_act):
            """Emit matmuls+bias for `groups` of x^T chunk `nch`, then the v
            transposes those groups enable (after, so the independent group
            matmuls cover the bias-add latency on the PE queue)."""
            for g in groups:
                M = 128 if g < 4 else 64
                ps = s_tile("qkv_ps")
                for ko in range(KO):
                    nc.tensor.matmul(
                        ps[:M, 0:512],
                        w_sb[:, ko, g * 128:g * 128 + M],
                        xt_sb[:, ko, 512 * nch:512 * (nch + 1)],
                        start=(ko == 0), stop=(ko == KO - 1),
                    )
                dst = qkvT[:M, g, 512 * nch:512 * (nch + 1)]
                if off_act or (g + nch) % 2:
                    nc.vector.tensor_scalar_add(dst, ps[:M, 0:512],
                                                b_sb[:M, g:g + 1])
                else:
                    nc.scalar.activation(dst, ps[:M, 0:512], AF.Identity,
                                         bias=b_sb[:M, g:g + 1], scale=1.0)
                yield
            for kt in range(4 * nch, 4 * nch + 4):
                if 2 in groups:
                    transpose_v_kt(0, kt, kt)
                    transpose_v_kt(1, kt, kt + 1)
                if 3 in groups:
                    transpose_v_kt(2, kt, kt)
                yield

        # -- attention: chunk j covers q columns 512j..512(j+1); k-tiles
        #    paired two per PSUM tile so each exp is one big ACT op --------
        def attn_chunk(h, j):
            qg, qb = Q_POS[h]
            kg, kb = K_POS[h]
            qT = qkvT[qb:qb + 64, qg, :]
            kT = qkvT[kb:kb + 64, kg, :]
            O_t = psp.tile([65, 512], F32, name=f"O_{h}_{j}", tag="O", bufs=O_BUFS)
            n_i = 4 * j + 4          # k-tiles contributing to this chunk
            for i in range(n_i):
                cs = max(128 * i, 512 * j)
                ce = 512 * (j + 1)
                w = ce - cs
                sp = s_tile(f"s_{h}_{j}_{i}")
                E = sbE.tile([128, 512], F32R, name="E", tag="E")
                nc.tensor.matmul(
                    sp[:, 0:w],
                    kT[:, 128 * i:128 * (i + 1)],
                    qT[:, cs:ce],
                    start=True, stop=True,
                )
                nc.scalar.activation(E[:, 0:w], sp[:, 0:w], AF.Exp, scale=0.125)
                if cs == 128 * i:
                    # diagonal tile: zero exp(S) where k > q (strictly lower
                    # triangle of the 128-wide diagonal block)
                    nc.vector.tensor_tensor(E[:, 0:128], E[:, 0:128], tri,
                                            ALU.mult)
                nc.tensor.matmul(
                    O_t[:, cs - 512 * j:cs - 512 * j + w],
                    v_sb[h][:, i, :],
                    E[:, 0:w],
                    start=(i == 0), stop=(i == n_i - 1),
                )
                yield
            # normalize O^T rows 0..63 by row 64 (the exp-sum)
            recip = sbN.tile([1, 512], F32, name="recip", tag="recip")
            nc.vector.reciprocal(recip, O_t[64:65, :])
            bc = sbN.tile([64, 512], F32, name="bc", tag="bc")
            nc.gpsimd.partition_broadcast(bc, recip, channels=64)
            if h == 0:
                dst = OT_a[0:64, 512 * j:512 * (j + 1)]
            elif h == 1:
                dst = h1tmp[:, 512 * j:512 * (j + 1)]
            else:
                dst = OT_b[:, 512 * j:512 * (j + 1)]
            nc.vector.tensor_tensor(dst, O_t[0:64, :], bc, ALU.mult)
            if h == 1:
                # head 1 lives on partitions 64..127 of the proj stationary
                nc.sync.dma_start(OT_a[64:128, 512 * j:512 * (j + 1)],
                                  h1tmp[:, 512 * j:512 * (j + 1)])

        def proj_tile(m):
            qsl = slice(128 * m, 128 * (m + 1))
            ya = s_tile("ya")
            yb = s_tile("yb")
            nc.tensor.matmul(ya[:, 0:512], OT_a[:, qsl], wpa[:, 0:512],
                             start=True, stop=False)
            nc.tensor.matmul(ya[:, 0:512], OT_b[:, qsl], wpb[:, 0:512],
                             start=False, stop=True)
            nc.tensor.matmul(yb[:, 0:256], OT_a[:, qsl], wpa[:, 512:768],
                             start=True, stop=False)
            nc.tensor.matmul(yb[:, 0:256], OT_b[:, qsl], wpb[:, 512:768],
                             start=False, stop=True)
            ysb = sbY.tile([128, C], BF16, name="ysb", tag="ysb")
            nc.vector.tensor_copy(ysb[:, 0:512], ya[:, 0:512])
            nc.scalar.copy(ysb[:, 512:768], yb[:, 0:256])
            nc.sync.dma_start(y_d[qsl, :], ysb)

        def interleave(gens):
            live = list(gens)
            while live:
                nxt = []
                for g in live:
                    try:
                        next(g)
                        nxt.append(g)
                    except StopIteration:
                        pass
                live = nxt

        def proj_chunk(c):
            for m in range(4 * c, 4 * c + 4):
                proj_tile(m)
                yield

        # round j runs: h0/h1 attention on q-chunk j, h2 attention on chunk
        # j-1, proj on chunk j-2, qkv groups 3-4 of x^T chunk j and groups
        # 0-2 of chunk j+1 (so every consumer's data is emitted a full round
        # before its in-order engine queue can reach it)
        if 'qkv' in phases:
            interleave([qkv_groups(0, (0, 1, 2), off_act=False)])
        for j in range(4):
            gens = []
            if 'attn' in phases:
                gens += [attn_chunk(0, j), attn_chunk(1, j)]
                if j >= 1:
                    gens.append(attn_chunk(2, j - 1))
                if 'proj' in phases and j >= 2:
                    gens.append(proj_chunk(j - 2))
            if 'qkv' in phases:
                off = 'attn' in phases
                gens.append(qkv_groups(j, (3, 4), off_act=off))
                if j < 3:
                    gens.append(qkv_groups(j + 1, (0, 1, 2), off_act=off))
            interleave(gens)
        if 'attn' in phases:
            gens = [attn_chunk(2, 3)]
            if 'proj' in phases:
                gens.append(proj_chunk(2))
            interleave(gens)
            if 'proj' in phases:
                interleave([proj_chunk(3)])
        if 'proj' not in phases:
            # stand-in output writeback so every variant writes y identically
            for m in range(16):
                ysb = sbY.tile([128, C], BF16, name="ysb", tag="ysb")
                nc.vector.memset(ysb, 0.0)
                nc.sync.dma_start(y_d[128 * m:128 * (m + 1), :], ysb)


def build_module(loop_n=1, phases=('qkv', 'attn', 'proj')):
    nc = bacc.Bacc()
    xt_d = nc.declare_dram_parameter("xt", [C, T], F32R, isOutput=False)
    w_d = nc.declare_dram_parameter("wqkv", [C, 576], F32R, isOutput=False)
    b_d = nc.declare_dram_parameter("bqkv", [128, 5], F32, isOutput=False)
    wp_d = nc.declare_dram_parameter("wp", [192, C], F32R, isOutput=False)
    y_d = nc.declare_dram_parameter("y", [T, C], BF16, isOutput=True)
    with tile.TileContext(nc) as tc:
        if loop_n > 1:
            with tc.For_i(0, loop_n, 1):
                _build_body(nc, tc, xt_d, w_d, b_d, wp_d, y_d, phases)
        else:
            _build_body(nc, tc, xt_d, w_d, b_d, wp_d, y_d, phases)
    nc.compile()
    return nc


def make_in_maps(x, W_attn, b_attn, W_proj):
    """Shard full inputs into the 8 per-core input maps."""
    x = np.asarray(x, np.float32)
    W_attn = np.asarray(W_attn, np.float32)
    b_attn = np.asarray(b_attn, np.float32)
    W_proj = np.asarray(W_proj, np.float32)
    xts = [to_f32r(x[b].T) for b in range(B)]
    in_maps = []
    for c in range(NCORES):
        b = c // (NCORES // B)
        heads = [(c % (NCORES // B)) * HPC + j for j in range(HPC)]
        cols, bias = [], []
        for kind, hi in W_ORDER:
            lo = kind * C + heads[hi] * DH
            cols.append(W_attn[:, lo:lo + DH])
            bias.append(b_attn[lo:lo + DH])
        wqkv = np.ascontiguousarray(np.concatenate(cols, axis=1))
        bq = np.concatenate(bias + [np.zeros(64, np.float32)])
        bq = np.ascontiguousarray(bq.reshape(5, 128).T)
        wp = np.concatenate(
            [W_proj[hh * DH:(hh + 1) * DH, :] for hh in heads], axis=0)
        in_maps.append({"xt": xts[b], "wqkv": to_f32r(wqkv),
                        "bqkv": bq, "wp": to_f32r(wp)})
    return in_maps


_module_cache = {}


def kernel(x, W_attn, b_attn, W_proj, b_proj):
    if "nc" not in _module_cache:
        _module_cache["nc"] = build_module()
    nc = _module_cache["nc"]
    in_maps = make_in_maps(x, W_attn, b_attn, W_proj)
    res = run_bass_kernel_spmd(nc, in_maps, core_ids=list(range(NCORES)))
    y = np.zeros((B, T, C), np.float64)
    for c in range(NCORES):
        y[c // (NCORES // B)] += res.results[c]["y"].astype(np.float64)
    y += np.asarray(b_proj, np.float64)
    return y.astype(np.float32)


# revision 65
# speedup vs baseline: 1.1458x; 1.1458x over previous
"""Causal self-attention (B=2, T=2048, C=768, H=12) on 8 TRN2 NeuronCores.

Sharding: 24 (batch, head) pairs -> 8 cores x 3 heads (head-tensor-parallel
within a batch, data-parallel across the 2 batches: cores 0-3 = batch 0,
cores 4-7 = batch 1). Each core computes qkv for its 3 heads, causal
attention, and a rank-192 partial of the output projection; the host sums
the 4 partials per batch and adds b_proj.

Per-core kernel (matmuls in float32r = full-rate PE, inputs pre-rounded on
the host; this runtime miscompiles every op with 2-byte input + 4-byte
output — bf16 matmuls, bf16->f32 copies, XBAR dma transpose — so bf16 is
used only for the y writeback, where the f32->bf16 downcast write is safe):

  - x^T [768,2048] streamed against W-slices -> qkv transposed [576,2048];
    w and x^T chunk 0 DMA per contraction tile so the first matmul starts
    ~1.7us in; scores S^T[k,q] per 128-row k-tile so softmax needs no max
    subtraction and O^T accumulates over k-tiles in PSUM; the softmax
    denominator falls out of the same matmul via a ones-column appended to
    the v stationary (65th output row); causal mask = DVE multiply by a
    precomputed 0/1 triangle on diagonal tiles only.
  - the whole kernel is one software pipeline over 512-column q-chunks:
      pre:       qkv chunk 0 (all groups) + its v transposes (DMA-paced)
      round j:   attn(h0,j) + attn(h1,j) + attn(h2,j-1) + proj(chunk j-2)
                 + qkv groups 3-4 of chunk j (j>=1) + groups 0-2 of chunk j+1
      tail:      attn(h2,3), proj chunks 2-3
    emitted via interleaved generators; every consumer's producers are
    emitted at least one round earlier so the in-order engine queues never
    deadlock and the exp stream (ACT) starts ~8us into the kernel.
  - engine placement: exps on ACT; masks, v copies, normalize, qkv bias on
    DVE (gpsimd cannot touch PSUM); ysb staging split DVE/ACT.
  - PSUM: S-tag 5x[128,512] (qkv, scores, transposes, proj) + O 3x1 bank.
"""

import os
import sys

for _p in ("/opt/trn_rl_repo", "/root/.axon_site/_ro/trn_rl_repo"):
    if os.path.isdir(_p) and _p not in sys.path:
        sys.path.insert(0, _p)

import numpy as np

import concourse.bass as bass  # noqa: F401
import concourse.mybir as mybir
import concourse.tile as tile
from concourse import bacc
from concourse.bass_utils import run_bass_kernel_spmd
from concourse.masks import make_identity

B, T, C, H, DH = 2, 2048, 768, 12, 64
HPC = 3          # heads per core
NCORES = 8
KO = C // 128    # 6 contraction tiles over the model dim
F32 = mybir.dt.float32
F32R = mybir.dt.float32r
BF16 = mybir.dt.bfloat16
AF = mybir.ActivationFunctionType
ALU = mybir.AluOpType
E_BUFS, N_BUFS, Y_BUFS, S_BUFS, O_BUFS = 8, 4, 4, 5, 3
WARMUP_MM = 6

# qkv row groups: G0=[q0|q1] G1=[k0|k1] G2=[v0|v1] G3=[q2|v2] G4=[k2|pad]
# (matmul requires lhsT/rhs at the same partition base, so each head's q and
# k must share a base: h0/h2 at base 0, h1 at base 64)
Q_POS = {0: (0, 0), 1: (0, 64), 2: (3, 0)}
K_POS = {0: (1, 0), 1: (1, 64), 2: (4, 0)}
V_POS = {0: (2, 0), 1: (2, 64), 2: (3, 64)}
# host-side column order matching the groups ((kind 0=q/1=k/2=v, head idx))
W_ORDER = [(0, 0), (0, 1), (1, 0), (1, 1), (2, 0), (2, 1), (0, 2), (2, 2),
           (1, 2)]


def to_f32r(a):
    """Round fp32 to the fp32r wire format: 8-bit exponent, 11-bit mantissa
    (round-to-nearest-even), low 12 mantissa bits zero. Matches walrus's
    fp32_to_fp32r; required for data consumed directly by fp32r matmuls."""
    u = np.ascontiguousarray(a, np.float32).view(np.uint32).copy()
    low = u & np.uint32(0xFFF)
    lsb = (u >> np.uint32(12)) & np.uint32(1)
    add = ((low > 0x800) | ((low == 0x800) & (lsb == 1))).astype(np.uint32) << 12
    u = ((u & ~np.uint32(0xFFF)) + add).astype(np.uint32)
    return u.view(np.float32)


def _build_body(nc, tc, xt_d, w_d, b_d, wp_d, y_d, phases=('qkv', 'attn', 'proj')):
    with (
        tc.tile_pool(name="persist", bufs=1) as pp,
        tc.tile_pool(name="sb_att", bufs=E_BUFS) as sbE,
        tc.tile_pool(name="sb_n", bufs=N_BUFS) as sbN,
        tc.tile_pool(name="sb_y", bufs=Y_BUFS) as sbY,
        tc.tile_pool(name="psum", bufs=1, space="PSUM") as psp,
    ):
        # -- constants ----------------------------------------------------
        ident = pp.tile([128, 128], F32, name="ident")
        make_identity(nc, ident)
        # causal triangle: tri[p, c] = 1 where c >= p else 0 (applied to the
        # 128-wide diagonal blocks of exp(S^T) by a DVE multiply)
        tri = pp.tile([128, 128], F32, name="tri")
        nc.gpsimd.memset(tri, 1.0)
        nc.gpsimd.affine_select(
            out=tri, in_=tri, compare_op=ALU.is_ge, fill=0.0,
            base=0, pattern=[[1, 128]], channel_multiplier=-1,
        )

        # -- PE warmup: keep the HAM activity monitor busy while the input
        # DMAs land so real matmuls start at 2.4 GHz, not 1.2 --------------
        warm = psp.tile([128, 512], F32, name="warm", tag="S", bufs=S_BUFS)
        for wi in range(WARMUP_MM):
            nc.tensor.matmul(warm[:, 0:128], ident, ident,
                             start=True, stop=True, skip_group_check=True)

        # -- input DMAs (bias+weights first -- the first qkv matmul needs
        # w_sb and xt chunk 0; wp last: only needed by the projection) ----
        b_sb = pp.tile([128, 5], F32, name="b_sb")
        nc.sync.dma_start(b_sb[:], b_d[:])
        # w and x^T chunk 0 split per contraction tile so the first qkv
        # matmul chain can start after ~1us instead of waiting for both
        # tensors in full; later x^T chunks land well ahead of use
        w_sb = pp.tile([128, KO, 576], F32R, name="w_sb")
        w_r = w_d.rearrange("(ko p) m -> p ko m", p=128)
        xt_sb = pp.tile([128, KO, T], F32R, name="xt_sb")
        xt_r = xt_d.rearrange("(ko p) n -> p ko n", p=128)
        for ko in range(KO):
            nc.sync.dma_start(w_sb[:, ko, :], w_r[:, ko, :])
            nc.sync.dma_start(xt_sb[:, ko, 0:512], xt_r[:, ko, 0:512])
        for nch in range(1, 4):
            nc.sync.dma_start(
                xt_sb[:, :, 512 * nch:512 * (nch + 1)],
                xt_r[:, :, 512 * nch:512 * (nch + 1)],
            )
        wpa = pp.tile([128, C], F32R, name="wpa")
        nc.sync.dma_start(wpa, wp_d[0:128, :])
        wpb = pp.tile([64, C], F32R, name="wpb")
        nc.sync.dma_start(wpb, wp_d[128:192, :])

        qkvT = pp.tile([128, 5, T], F32R, name="qkvT")
        ones_col = pp.tile([128, 1], F32R, name="ones_col")
        nc.scalar.activation(ones_col, b_sb[:, 0:1], AF.Copy, bias=1.0, scale=0.0)
        v_sb = [pp.tile([128, 16, 65], F32R, name=f"v_sb{h}") for h in range(HPC)]
        for h in range(HPC):
            nc.vector.tensor_copy(v_sb[h][:, :, 64:65],
                                  ones_col.broadcast_to([128, 16, 1]))
        OT_a = pp.tile([128, T], F32R, name="OT_a")   # heads 0,1 of O^T
        OT_b = pp.tile([64, T], F32R, name="OT_b")    # head 2
        h1tmp = pp.tile([64, T], F32R, name="h1tmp")  # head 1 staging

        def s_tile(name):
            # [128,512] f32 = 1 PSUM bank; S 5 + O 3 = all 8 banks
            return psp.tile([128, 512], F32, name=name, tag="S", bufs=S_BUFS)

        # -- qkv: per 512-col x^T chunk, all 5 groups + the v transposes
        #    that chunk enables; emitted as a generator for interleaving --
        def transpose_v01_kt(kt):
            # g2 holds [v0 | v1] on partitions 0:64 / 64:128 -> one 128-wide
            # transpose yields both heads' v tiles side by side
            vT = qkvT[0:128, 2, 128 * kt:128 * (kt + 1)]
            tp = s_tile("tp_ps")
            nc.tensor.transpose(tp[:, 0:128], vT.bitcast(F32), ident)
            # gpsimd cannot read PSUM: v copies stay on DVE
            nc.vector.tensor_copy(v_sb[0][:, kt, 0:64], tp[:, 0:64])
            nc.vector.tensor_copy(v_sb[1][:, kt, 0:64], tp[:, 64:128])

        def transpose_v2_kt(kt):
            g, r0 = V_POS[2]
            vT = qkvT[r0:r0 + 64, g, 128 * kt:128 * (kt + 1)]
            tp = s_tile("tp_ps")
            nc.tensor.transpose(tp[:, 0:64], vT.bitcast(F32),
                                ident[r0:r0 + 64, r0:r0 + 64])
            nc.vector.tensor_copy(v_sb[2][:, kt, 0:64], tp[:, 0:64])

        def qkv_groups(nch, groups, off_act):
            """Emit matmuls+bias for `groups` of x^T chunk `nch`, then the v
            transposes those groups enable (after, so the independent group
            matmuls cover the bias-add latency on the PE queue)."""
            for g in groups:
                M = 128 if g < 4 else 64
                ps = s_tile("qkv_ps")
                for ko in range(KO):
                    nc.tensor.matmul(
                        ps[:M, 0:512],
                        w_sb[:, ko, g * 128:g * 128 + M],
                        xt_sb[:, ko, 512 * nch:512 * (nch + 1)],
                        start=(ko == 0), stop=(ko == KO - 1),
                    )
                dst = qkvT[:M, g, 512 * nch:512 * (nch + 1)]
                if off_act or (g + nch) % 2:
                    nc.vector.tensor_scalar_add(dst, ps[:M, 0:512],
                                                b_sb[:M, g:g + 1])
                else:
                    nc.scalar.activation(dst, ps[:M, 0:512], AF.Identity,
                                         bias=b_sb[:M, g:g + 1], scale=1.0)
                yield
            for kt in range(4 * nch, 4 * nch + 4):
                if 2 in groups:
                    transpose_v01_kt(kt)
                if 3 in groups:
                    transpose_v2_kt(kt)
                yield

        # -- attention: chunk j covers q columns 512j..512(j+1); one S PSUM
        #    bank + one exp per 128-row k-tile -----------------------------
        def attn_chunk(h, j):
            qg, qb = Q_POS[h]
            kg, kb = K_POS[h]
            qT = qkvT[qb:qb + 64, qg, :]
            # flattened view for padded reads past a chunk boundary (the
            # neighbouring group's data -- valid, written memory; the padded
            # output columns are never exp'd or read)
            qTf = qkvT[qb:qb + 64].rearrange("p g n -> p (g n)")
            kT = qkvT[kb:kb + 64, kg, :]
            O_t = psp.tile([65, 512], F32, name=f"O_{h}_{j}", tag="O", bufs=O_BUFS)
            n_i = 4 * j + 4          # k-tiles contributing to this chunk
            pend = []                # (i, cs, w, E) with exp in flight

            def o_mm(i, cs, w, E):
                nc.tensor.matmul(
                    O_t[:, cs - 512 * j:cs - 512 * j + w],
                    v_sb[h][:, i, :],
                    E[:, 0:w],
                    start=(i == 0), stop=(i == n_i - 1),
                )

            # software-pipelined: each O matmul is emitted one k-tile after
            # its exp, so the PE has an independent S matmul to chew on
            # while the ACT engine finishes exp(i)
            for i in range(n_i):
                cs = max(128 * i, 512 * j)
                ce = 512 * (j + 1)
                w = ce - cs
                sp = s_tile(f"s_{h}_{j}_{i}")
                E = sbE.tile([128, 512], F32R, name="E", tag="E")
                if w == 128:
                    # pad the score matmul to 256 columns: fp32r runs 4x
                    # slower below 256-wide; the extra columns land in
                    # sp[:, 128:256] and are never read
                    nc.tensor.matmul(
                        sp[:, 0:256],
                        kT[:, 128 * i:128 * (i + 1)],
                        qTf[:, 2048 * qg + cs:2048 * qg + cs + 256],
                        start=True, stop=True,
                    )
                else:
                    nc.tensor.matmul(
                        sp[:, 0:w],
                        kT[:, 128 * i:128 * (i + 1)],
                        qT[:, cs:ce],
                        start=True, stop=True,
                    )
                nc.scalar.activation(E[:, 0:w], sp[:, 0:w], AF.Exp, scale=0.125)
                if cs == 128 * i:
                    # diagonal tile: zero exp(S) where k > q (strictly lower
                    # triangle of the 128-wide diagonal block)
                    nc.vector.tensor_tensor(E[:, 0:128], E[:, 0:128], tri,
                                            ALU.mult)
                if len(pend) >= 2:
                    o_mm(*pend.pop(0))
                pend.append((i, cs, w, E))
                yield
            for t in pend:
                o_mm(*t)
            # normalize O^T rows 0..63 by row 64 (the exp-sum)
            recip = sbN.tile([1, 512], F32, name="recip", tag="recip")
            nc.vector.reciprocal(recip, O_t[64:65, :])
            bc = sbN.tile([64, 512], F32, name="bc", tag="bc")
            nc.gpsimd.partition_broadcast(bc, recip, channels=64)
            if h == 0:
                dst = OT_a[0:64, 512 * j:512 * (j + 1)]
            elif h == 1:
                dst = h1tmp[:, 512 * j:512 * (j + 1)]
            else:
                dst = OT_b[:, 512 * j:512 * (j + 1)]
            nc.vector.tensor_tensor(dst, O_t[0:64, :], bc, ALU.mult)
            if h == 1:
                # head 1 lives on partitions 64..127 of the proj stationary
                nc.sync.dma_start(OT_a[64:128, 512 * j:512 * (j + 1)],
                                  h1tmp[:, 512 * j:512 * (j + 1)])

        def proj_tile(m):
            qsl = slice(128 * m, 128 * (m + 1))
            ya = s_tile("ya")
            yb = s_tile("yb")
            nc.tensor.matmul(ya[:, 0:512], OT_a[:, qsl], wpa[:, 0:512],
                             start=True, stop=False)
            nc.tensor.matmul(ya[:, 0:512], OT_b[:, qsl], wpb[:, 0:512],
                             start=False, stop=True)
            nc.tensor.matmul(yb[:, 0:256], OT_a[:, qsl], wpa[:, 512:768],
                             start=True, stop=False)
            nc.tensor.matmul(yb[:, 0:256], OT_b[:, qsl], wpb[:, 512:768],
                             start=False, stop=True)
            ysb = sbY.tile([128, C], BF16, name="ysb", tag="ysb")
            nc.vector.tensor_copy(ysb[:, 0:512], ya[:, 0:512])
            nc.scalar.copy(ysb[:, 512:768], yb[:, 0:256])
            nc.sync.dma_start(y_d[qsl, :], ysb)

        def interleave(gens):
            live = list(gens)
            while live:
                nxt = []
                for g in live:
                    try:
                        next(g)
                        nxt.append(g)
                    except StopIteration:
                        pass
                live = nxt

        def proj_chunk(c):
            for m in range(4 * c, 4 * c + 4):
                proj_tile(m)
                yield
                yield
                yield

        # round j runs: h0/h1 attention on q-chunk j, h2 attention on chunk
        # j-1, proj on chunk j-2, qkv groups 3-4 of x^T chunk j and groups
        # 0-2 of chunk j+1 (so every consumer's data is emitted a full round
        # before its in-order engine queue can reach it)
        if 'qkv' in phases:
            interleave([qkv_groups(0, (0, 1, 2, 3, 4), off_act=False)])
        for j in range(4):
            gens = []
            if 'attn' in phases:
                gens.append(attn_chunk(0, j))
            if 'qkv' in phases and j >= 1:
                gens.append(qkv_groups(j, (3, 4),
                                       off_act='attn' in phases and j >= 2))
            if 'attn' in phases:
                gens.append(attn_chunk(1, j))
            if 'qkv' in phases and j < 3:
                gens.append(qkv_groups(j + 1, (0, 1, 2),
                                       off_act='attn' in phases and j >= 1))
            if 'attn' in phases:
                if j >= 1:
                    gens.append(attn_chunk(2, j - 1))
                if 'proj' in phases and j >= 2:
                    gens.append(proj_chunk(j - 2))
            interleave(gens)
        if 'attn' in phases:
            gens = [attn_chunk(2, 3)]
            if 'proj' in phases:
                gens.append(proj_chunk(2))
            interleave(gens)
            if 'proj' in phases:
                interleave([proj_chunk(3)])
        if 'proj' not in phases:
            # stand-in output writeback so every variant writes y identically
            for m in range(16):
                ysb = sbY.tile([128, C], BF16, name="ysb", tag="ysb")
                nc.vector.memset(ysb, 0.0)
                nc.sync.dma_start(y_d[128 * m:128 * (m + 1), :], ysb)


def build_module(loop_n=1, phases=('qkv', 'attn', 'proj')):
    nc = bacc.Bacc()
    xt_d = nc.declare_dram_parameter("xt", [C, T], F32R, isOutput=False)
    w_d = nc.declare_dram_parameter("wqkv", [C, 576], F32R, isOutput=False)
    b_d = nc.declare_dram_parameter("bqkv", [128, 5], F32, isOutput=False)
    wp_d = nc.declare_dram_parameter("wp", [192, C], F32R, isOutput=False)
    y_d = nc.declare_dram_parameter("y", [T, C], BF16, isOutput=True)
    with tile.TileContext(nc) as tc:
        if loop_n > 1:
            with tc.For_i(0, loop_n, 1):
                _build_body(nc, tc, xt_d, w_d, b_d, wp_d, y_d, phases)
        else:
            _build_body(nc, tc, xt_d, w_d, b_d, wp_d, y_d, phases)
    nc.compile()
    return nc


def make_in_maps(x, W_attn, b_attn, W_proj):
    """Shard full inputs into the 8 per-core input maps."""
    x = np.asarray(x, np.float32)
    W_attn = np.asarray(W_attn, np.float32)
    b_attn = np.asarray(b_attn, np.float32)
    W_proj = np.asarray(W_proj, np.float32)
    xts = [to_f32r(x[b].T) for b in range(B)]
    in_maps = []
    for c in range(NCORES):
        b = c // (NCORES // B)
        heads = [(c % (NCORES // B)) * HPC + j for j in range(HPC)]
        cols, bias = [], []
        for kind, hi in W_ORDER:
            lo = kind * C + heads[hi] * DH
            cols.append(W_attn[:, lo:lo + DH])
            bias.append(b_attn[lo:lo + DH])
        wqkv = np.ascontiguousarray(np.concatenate(cols, axis=1))
        bq = np.concatenate(bias + [np.zeros(64, np.float32)])
        bq = np.ascontiguousarray(bq.reshape(5, 128).T)
        wp = np.concatenate(
            [W_proj[hh * DH:(hh + 1) * DH, :] for hh in heads], axis=0)
        in_maps.append({"xt": xts[b], "wqkv": to_f32r(wqkv),
                        "bqkv": bq, "wp": to_f32r(wp)})
    return in_maps


_module_cache = {}


def kernel(x, W_attn, b_attn, W_proj, b_proj):
    if "nc" not in _module_cache:
        _module_cache["nc"] = build_module()
    nc = _module_cache["nc"]
    in_maps = make_in_maps(x, W_attn, b_attn, W_proj)
    res = run_bass_kernel_spmd(nc, in_maps, core_ids=list(range(NCORES)))
    y = np.zeros((B, T, C), np.float64)
    for c in range(NCORES):
        y[c // (NCORES // B)] += res.results[c]["y"].astype(np.float64)
    y += np.asarray(b_proj, np.float64)
    return y.astype(np.float32)
